# revision 13
# baseline (speedup 1.0000x reference)
"""MoE (E=128, top-8, cap=20) on 8 TRN2 NeuronCores, expert-parallel.

Host: router + dispatch-index computation (tiny, exact-match w/ reference
semantics via jax-on-CPU) and buffer permutation. Device: per-expert MLP
silu(x @ w1 + b1) @ w2 + b2, scaled by the per-slot combine weight, 16
experts per core, bf16 matmuls with fp32 accumulate. Host: scatter-add
combine + aux loss.
"""

import math
import os

os.environ.setdefault("JAX_PLATFORMS", "axon,cpu")
os.environ.setdefault("JAX_COMPILATION_CACHE_DIR", "/tmp/jaxcache")
os.environ.setdefault("JAX_PERSISTENT_CACHE_MIN_ENTRY_SIZE_BYTES", "0")
os.environ.setdefault("JAX_PERSISTENT_CACHE_MIN_COMPILE_TIME_SECS", "0")
if os.environ.get("JAX_PLATFORMS") == "axon":
    os.environ["JAX_PLATFORMS"] = "axon,cpu"

import numpy as np

E = 128
K = 8
CAP_FACTOR = 1.25
EPS = 1e-9
B, N, L2, C, H = 2048, 4, 16, 256, 512
CAP = int(math.ceil(CAP_FACTOR * B / E))  # 20
NCORES = 8
EPC = E // NCORES                         # 16 experts per core
ROWS = CAP * N * L2                       # 1280 rows per expert
NJ = ROWS // 128                          # 10 gemm2 row-chunks per expert
ROW_TILES = [(0, 512), (512, 512), (1024, 256)]

_NC_CACHE = {}


def _cpu_device():
    import jax

    try:
        return jax.devices("cpu")[0]
    except Exception:
        return None


class _MaybeCpu:
    """jax.default_device(cpu) when available, else no-op."""

    def __init__(self):
        self.dev = _cpu_device()

    def __enter__(self):
        import contextlib

        import jax

        self._cm = (
            jax.default_device(self.dev) if self.dev is not None
            else contextlib.nullcontext()
        )
        return self._cm.__enter__()

    def __exit__(self, *a):
        return self._cm.__exit__(*a)


def _routing(router_fea, router_w):
    """Exact reference router semantics on jax-CPU.

    Returns (tok_slot [E,CAP] i32, w_slot [E,CAP] f32, aux f32 scalar).
    """
    import jax
    import jax.numpy as jnp

    if _cpu_device() is None:
        return _routing_numpy(router_fea, router_w)

    with jax.default_device(_cpu_device()):
        rf = jnp.asarray(np.asarray(router_fea))
        rw = jnp.asarray(np.asarray(router_w))
        logits = rf @ rw
        probs = jax.nn.softmax(logits, axis=-1)
        topk_val, topk_idx = jax.lax.top_k(probs, K)
        topk_w = topk_val / (topk_val.sum(-1, keepdims=True) + EPS)

        flat_tok = jnp.repeat(jnp.arange(B, dtype=jnp.int32), K)
        flat_exp = topk_idx.reshape(-1).astype(jnp.int32)
        flat_w = topk_w.reshape(-1)

        order = jnp.argsort(-flat_w)
        exp_sorted = flat_exp[order]
        oh = jax.nn.one_hot(exp_sorted, E, dtype=jnp.int32)
        rank_sorted = (
            jnp.take_along_axis(jnp.cumsum(oh, axis=0), exp_sorted[:, None], 1)[:, 0]
            - 1
        )
        rank = jnp.zeros_like(rank_sorted).at[order].set(rank_sorted)
        kept = rank < CAP

        kept_mask = kept.reshape(B, K)
        kept_w = topk_w * kept_mask
        kept_w = kept_w / (kept_w.sum(-1, keepdims=True) + EPS)
        kept_w_flat = kept_w.reshape(-1)

        slot = jnp.where(kept, rank, CAP)
        tok_slot = (
            jnp.zeros((E, CAP + 1), jnp.int32).at[flat_exp, slot].set(flat_tok)[:, :CAP]
        )
        w_slot = (
            jnp.zeros((E, CAP + 1), kept_w_flat.dtype)
            .at[flat_exp, slot]
            .set(kept_w_flat)[:, :CAP]
        )

        assigned = (
            jnp.zeros((B, E), probs.dtype)
            .at[flat_tok, flat_exp]
            .add(kept.astype(probs.dtype))
        )
        importance = probs.sum(0)
        load = assigned.sum(0)
        imp_n = importance / (importance.sum() + EPS)
        load_n = load / (load.sum() + EPS)
        aux = E * (imp_n * load_n).sum()

    return np.asarray(tok_slot), np.asarray(w_slot), np.asarray(aux)


def _routing_numpy(router_fea, router_w):
    """Fallback router in numpy f32 (same semantics; used only if jax-CPU
    backend is unavailable in the calling process)."""
    rf = np.asarray(router_fea, dtype=np.float32)
    rw = np.asarray(router_w, dtype=np.float32)
    logits = rf @ rw
    m = logits.max(-1, keepdims=True)
    ex = np.exp(logits - m)
    probs = ex / ex.sum(-1, keepdims=True)
    topk_idx = np.argsort(-probs, axis=-1, kind="stable")[:, :K].astype(np.int32)
    topk_val = np.take_along_axis(probs, topk_idx, axis=-1)
    topk_w = topk_val / (topk_val.sum(-1, keepdims=True) + np.float32(EPS))
    flat_tok = np.repeat(np.arange(B, dtype=np.int32), K)
    flat_exp = topk_idx.reshape(-1)
    flat_w = topk_w.reshape(-1)
    order = np.argsort(-flat_w, kind="stable")
    rank = np.zeros(B * K, np.int32)
    counts = np.zeros(E, np.int32)
    for i in order:
        e_ = flat_exp[i]
        rank[i] = counts[e_]
        counts[e_] += 1
    kept = rank < CAP
    kept_w = (topk_w * kept.reshape(B, K)).astype(np.float32)
    kept_w = kept_w / (kept_w.sum(-1, keepdims=True) + np.float32(EPS))
    kept_w_flat = kept_w.reshape(-1)
    slot = np.where(kept, rank, CAP)
    tok_slot = np.zeros((E, CAP + 1), np.int32)
    w_slot = np.zeros((E, CAP + 1), np.float32)
    tok_slot[flat_exp, slot] = flat_tok
    w_slot[flat_exp, slot] = kept_w_flat
    tok_slot = tok_slot[:, :CAP]
    w_slot = w_slot[:, :CAP]
    assigned = np.zeros((B, E), np.float32)
    np.add.at(assigned, (flat_tok, flat_exp), kept.astype(np.float32))
    importance = probs.sum(0)
    load = assigned.sum(0)
    imp_n = importance / (importance.sum() + np.float32(EPS))
    load_n = load / (load.sum() + np.float32(EPS))
    aux = np.float32(E) * (imp_n * load_n).sum()
    return tok_slot, w_slot, np.asarray(aux, dtype=np.float32)


def _combine(dev_out, tok_slot):
    """Scatter-add device outputs [E,CAP,64,C] back to tokens via jax-CPU."""
    import jax
    import jax.numpy as jnp

    if _cpu_device() is None:
        y = np.zeros((B, N * L2, C), np.float32)
        np.add.at(y, tok_slot.reshape(-1), dev_out.reshape(E * CAP, N * L2, C))
        return y.reshape(B, N, L2, C)

    with jax.default_device(_cpu_device()):
        y = (
            jnp.zeros((B, N * L2, C), jnp.float32)
            .at[jnp.asarray(tok_slot.reshape(-1))]
            .add(jnp.asarray(dev_out.reshape(E * CAP, N * L2, C)))
        )
    return np.asarray(y).reshape(B, N, L2, C)


def _shard_inputs(node_embedding, w1, b1, w2, b2, tok_slot, w_slot):
    """Build per-core device input dicts (bf16 matmul operands)."""
    import ml_dtypes

    bf16 = ml_dtypes.bfloat16
    ne = np.ascontiguousarray(node_embedding, dtype=np.float32).reshape(B, N * L2, C)
    w1 = np.asarray(w1, dtype=np.float32)
    b1 = np.asarray(b1, dtype=np.float32)
    w2 = np.asarray(w2, dtype=np.float32)
    b2 = np.asarray(b2, dtype=np.float32)

    in_maps = []
    for c in range(NCORES):
        es = slice(c * EPC, (c + 1) * EPC)
        idx = tok_slot[es].reshape(-1)                       # [EPC*CAP]
        x = ne[idx]                                          # [EPC*CAP, 64, C]
        # [EPC, 128(part), ck, ROWS] with partition p = channel within chunk
        xt = (
            x.reshape(EPC, ROWS, 2, 128)
            .transpose(0, 3, 2, 1)
            .reshape(EPC, 128, 2 * ROWS)
            .astype(bf16)
        )
        # [EPC, 128(part=c within ck), ck*512 + h]
        w1t = (
            w1[es]
            .reshape(EPC, 2, 128, H)
            .transpose(0, 2, 1, 3)
            .reshape(EPC, 128, 2 * H)
            .astype(bf16)
        )
        # [EPC, 128(part=h within hk), hk*256 + c]
        w2t = (
            w2[es]
            .reshape(EPC, 4, 128, C)
            .transpose(0, 2, 1, 3)
            .reshape(EPC, 128, 4 * C)
            .astype(bf16)
        )
        b1t = np.ascontiguousarray(b1[es].reshape(EPC, 4, 128).transpose(0, 2, 1))
        in_maps.append(
            {
                "xt": np.ascontiguousarray(xt),
                "w1t": np.ascontiguousarray(w1t),
                "w2t": np.ascontiguousarray(w2t),
                "b1t": b1t,
            }
        )
    return in_maps


def _build_nc():
    """Per-core Bass program: 16-expert grouped MLP with combine-weight scale."""
    import concourse.bacc as bacc
    import concourse.mybir as mybir
    from concourse.tile import TileContext

    f32 = mybir.dt.float32
    bf16 = mybir.dt.bfloat16

    nc = bacc.Bacc()
    xt_d = nc.dram_tensor("xt", [EPC, 128, 2 * ROWS], bf16, kind="ExternalInput")
    w1_d = nc.dram_tensor("w1t", [EPC, 128, 2 * H], bf16, kind="ExternalInput")
    w2_d = nc.dram_tensor("w2t", [EPC, 128, 4 * C], bf16, kind="ExternalInput")
    b1_d = nc.dram_tensor("b1t", [EPC, 128, 4], f32, kind="ExternalInput")
    out_d = nc.dram_tensor("out", [EPC, 128, 2 * ROWS], bf16, kind="ExternalOutput")

    with TileContext(nc) as tc:
        with (
            tc.tile_pool(name="xp", bufs=3) as xp,
            tc.tile_pool(name="wp", bufs=3) as wp,
            tc.tile_pool(name="cp", bufs=3) as cp,
            tc.tile_pool(name="hp", bufs=2) as hp,
            tc.tile_pool(name="op", bufs=2) as op_,
            tc.tile_pool(name="pp1", bufs=5, space="PSUM") as pp1,
            tc.tile_pool(name="pp2", bufs=3, space="PSUM") as pp2,
        ):
            for e in range(EPC):
                xsb = xp.tile([128, 2 * ROWS], bf16, tag="x", name=f"x_{e}")
                w1sb = wp.tile([128, 2 * H], bf16, tag="w1", name=f"w1_{e}")
                w2sb = wp.tile([128, 4 * C], bf16, tag="w2", name=f"w2_{e}")
                if e == 0:
                    hw = ROWS // 2
                    nc.sync.dma_start(out=w1sb[:, :H], in_=w1_d[e][:, :H])
                    nc.sync.dma_start(out=xsb[:, :hw], in_=xt_d[e][:, :hw])
                    nc.sync.dma_start(out=w1sb[:, H:], in_=w1_d[e][:, H:])
                    nc.sync.dma_start(
                        out=xsb[:, ROWS : ROWS + hw], in_=xt_d[e][:, ROWS : ROWS + hw]
                    )
                    nc.sync.dma_start(out=xsb[:, hw:ROWS], in_=xt_d[e][:, hw:ROWS])
                    nc.sync.dma_start(
                        out=xsb[:, ROWS + hw :], in_=xt_d[e][:, ROWS + hw :]
                    )
                    nc.sync.dma_start(out=w2sb[:], in_=w2_d[e])
                else:
                    nc.sync.dma_start(out=w1sb[:], in_=w1_d[e])
                    nc.sync.dma_start(out=xsb[:], in_=xt_d[e])
                    nc.sync.dma_start(out=w2sb[:], in_=w2_d[e])
                b1s = cp.tile([128, 4], f32, tag="b1", name=f"b1_{e}")
                nc.sync.dma_start(out=b1s[:], in_=b1_d[e])

                osb = op_.tile([128, 2 * ROWS], bf16, tag="osb", name=f"osb_{e}")

                for t, (r0, rt) in enumerate(ROW_TILES):
                    ht = hp.tile([128, 4 * rt], bf16, tag="ht", name=f"ht_{e}_{t}")
                    for hk in range(4):
                        ps1 = pp1.tile(
                            [128, rt], f32, tag="ps1", name=f"ps1_{e}_{t}_{hk}"
                        )
                        for ck in range(2):
                            nc.tensor.matmul(
                                ps1[:],
                                lhsT=w1sb[:, ck * H + hk * 128 : ck * H + (hk + 1) * 128],
                                rhs=xsb[:, ck * ROWS + r0 : ck * ROWS + r0 + rt],
                                start=(ck == 0),
                                stop=(ck == 1),
                            )
                        nc.scalar.activation(
                            out=ht[:, hk * rt : (hk + 1) * rt],
                            in_=ps1[:],
                            func=mybir.ActivationFunctionType.Silu,
                            bias=b1s[:, hk : hk + 1],
                        )
                    # gemm2 transposed: out^T[c, rows] = w2^T @ h^T, bias+scale on host
                    for cc in range(2):
                        ps2 = pp2.tile([128, rt], f32, tag="ps2", name=f"ps2_{e}_{t}_{cc}")
                        for hk in range(4):
                            nc.tensor.matmul(
                                ps2[:],
                                lhsT=w2sb[:, hk * C + cc * 128 : hk * C + (cc + 1) * 128],
                                rhs=ht[:, hk * rt : (hk + 1) * rt],
                                start=(hk == 0),
                                stop=(hk == 3),
                            )
                        nc.vector.tensor_copy(
                            osb[:, cc * ROWS + r0 : cc * ROWS + r0 + rt], ps2[:]
                        )
                        if e == EPC - 1:
                            nc.sync.dma_start(
                                out=out_d[e][:, cc * ROWS + r0 : cc * ROWS + r0 + rt],
                                in_=osb[:, cc * ROWS + r0 : cc * ROWS + r0 + rt],
                            )
                if e < EPC - 1:
                    nc.sync.dma_start(out=out_d[e][:, :ROWS], in_=osb[:, :ROWS])
                    nc.sync.dma_start(out=out_d[e][:, ROWS:], in_=osb[:, ROWS:])

    nc.finalize()
    return nc


def _get_nc():
    if "nc" not in _NC_CACHE:
        _NC_CACHE["nc"] = _build_nc()
    return _NC_CACHE["nc"]


def _run_device(in_maps, trace=False, tmpdir=None):
    from concourse.bass_utils import run_bass_kernel_spmd

    nc = _get_nc()
    try:
        res = run_bass_kernel_spmd(
            nc, in_maps, list(range(NCORES)), trace=trace, tmpdir=tmpdir
        )
    except (ImportError, ModuleNotFoundError):
        # BASS_TRACE set in the environment but the axon NTFF hook module is
        # absent -- rerun with tracing hard-disabled.
        os.environ["BASS_NEVER_TRACE"] = "1"
        res = run_bass_kernel_spmd(
            nc, in_maps, list(range(NCORES)), trace=False, tmpdir=tmpdir
        )
    return res


def kernel(node_embedding, router_fea, router_w, w1, b1, w2, b2):
    tok_slot, w_slot, aux = _routing(router_fea, router_w)
    in_maps = _shard_inputs(node_embedding, w1, b1, w2, b2, tok_slot, w_slot)
    res = _run_device(in_maps)
    b2f = np.asarray(b2, dtype=np.float32)
    wfull = np.repeat(w_slot, N * L2, axis=1)  # [E, ROWS]
    parts = []
    for c in range(NCORES):
        o = (
            res.results[c]["out"]
            .astype(np.float32)
            .reshape(EPC, 128, 2, ROWS)
            .transpose(0, 3, 2, 1)
            .reshape(EPC, ROWS, C)
        )  # [EPC, ROWS, C] raw h@w2 (no bias/scale)
        es = slice(c * EPC, (c + 1) * EPC)
        o = (o + b2f[es][:, None, :]) * wfull[es][:, :, None]
        parts.append(o.reshape(EPC, CAP, N * L2, C))
    dev_out = np.concatenate(parts, axis=0)
    y = _combine(dev_out, tok_slot)
    return y, aux


# revision 15
# speedup vs baseline: 1.0310x; 1.0310x over previous
"""MoE (E=128, top-8, cap=20) on 8 TRN2 NeuronCores, expert-parallel.

Host: router + dispatch-index computation (tiny, exact-match w/ reference
semantics via jax-on-CPU) and buffer permutation. Device: per-expert MLP
silu(x @ w1 + b1) @ w2 + b2, scaled by the per-slot combine weight, 16
experts per core, bf16 matmuls with fp32 accumulate. Host: scatter-add
combine + aux loss.
"""

import math
import os

os.environ.setdefault("JAX_PLATFORMS", "axon,cpu")
os.environ.setdefault("JAX_COMPILATION_CACHE_DIR", "/tmp/jaxcache")
os.environ.setdefault("JAX_PERSISTENT_CACHE_MIN_ENTRY_SIZE_BYTES", "0")
os.environ.setdefault("JAX_PERSISTENT_CACHE_MIN_COMPILE_TIME_SECS", "0")
if os.environ.get("JAX_PLATFORMS") == "axon":
    os.environ["JAX_PLATFORMS"] = "axon,cpu"

import numpy as np

E = 128
K = 8
CAP_FACTOR = 1.25
EPS = 1e-9
B, N, L2, C, H = 2048, 4, 16, 256, 512
CAP = int(math.ceil(CAP_FACTOR * B / E))  # 20
NCORES = 8
EPC = E // NCORES                         # 16 experts per core
ROWS = CAP * N * L2                       # 1280 rows per expert
NJ = ROWS // 128                          # 10 gemm2 row-chunks per expert
ROW_TILES = [(0, 512), (512, 512), (1024, 256)]

_NC_CACHE = {}


def _cpu_device():
    import jax

    try:
        return jax.devices("cpu")[0]
    except Exception:
        return None


class _MaybeCpu:
    """jax.default_device(cpu) when available, else no-op."""

    def __init__(self):
        self.dev = _cpu_device()

    def __enter__(self):
        import contextlib

        import jax

        self._cm = (
            jax.default_device(self.dev) if self.dev is not None
            else contextlib.nullcontext()
        )
        return self._cm.__enter__()

    def __exit__(self, *a):
        return self._cm.__exit__(*a)


def _routing(router_fea, router_w):
    """Exact reference router semantics on jax-CPU.

    Returns (tok_slot [E,CAP] i32, w_slot [E,CAP] f32, aux f32 scalar).
    """
    import jax
    import jax.numpy as jnp

    if _cpu_device() is None:
        return _routing_numpy(router_fea, router_w)

    with jax.default_device(_cpu_device()):
        rf = jnp.asarray(np.asarray(router_fea))
        rw = jnp.asarray(np.asarray(router_w))
        logits = rf @ rw
        probs = jax.nn.softmax(logits, axis=-1)
        topk_val, topk_idx = jax.lax.top_k(probs, K)
        topk_w = topk_val / (topk_val.sum(-1, keepdims=True) + EPS)

        flat_tok = jnp.repeat(jnp.arange(B, dtype=jnp.int32), K)
        flat_exp = topk_idx.reshape(-1).astype(jnp.int32)
        flat_w = topk_w.reshape(-1)

        order = jnp.argsort(-flat_w)
        exp_sorted = flat_exp[order]
        oh = jax.nn.one_hot(exp_sorted, E, dtype=jnp.int32)
        rank_sorted = (
            jnp.take_along_axis(jnp.cumsum(oh, axis=0), exp_sorted[:, None], 1)[:, 0]
            - 1
        )
        rank = jnp.zeros_like(rank_sorted).at[order].set(rank_sorted)
        kept = rank < CAP

        kept_mask = kept.reshape(B, K)
        kept_w = topk_w * kept_mask
        kept_w = kept_w / (kept_w.sum(-1, keepdims=True) + EPS)
        kept_w_flat = kept_w.reshape(-1)

        slot = jnp.where(kept, rank, CAP)
        tok_slot = (
            jnp.zeros((E, CAP + 1), jnp.int32).at[flat_exp, slot].set(flat_tok)[:, :CAP]
        )
        w_slot = (
            jnp.zeros((E, CAP + 1), kept_w_flat.dtype)
            .at[flat_exp, slot]
            .set(kept_w_flat)[:, :CAP]
        )

        assigned = (
            jnp.zeros((B, E), probs.dtype)
            .at[flat_tok, flat_exp]
            .add(kept.astype(probs.dtype))
        )
        importance = probs.sum(0)
        load = assigned.sum(0)
        imp_n = importance / (importance.sum() + EPS)
        load_n = load / (load.sum() + EPS)
        aux = E * (imp_n * load_n).sum()

    return np.asarray(tok_slot), np.asarray(w_slot), np.asarray(aux)


def _routing_numpy(router_fea, router_w):
    """Fallback router in numpy f32 (same semantics; used only if jax-CPU
    backend is unavailable in the calling process)."""
    rf = np.asarray(router_fea, dtype=np.float32)
    rw = np.asarray(router_w, dtype=np.float32)
    logits = rf @ rw
    m = logits.max(-1, keepdims=True)
    ex = np.exp(logits - m)
    probs = ex / ex.sum(-1, keepdims=True)
    topk_idx = np.argsort(-probs, axis=-1, kind="stable")[:, :K].astype(np.int32)
    topk_val = np.take_along_axis(probs, topk_idx, axis=-1)
    topk_w = topk_val / (topk_val.sum(-1, keepdims=True) + np.float32(EPS))
    flat_tok = np.repeat(np.arange(B, dtype=np.int32), K)
    flat_exp = topk_idx.reshape(-1)
    flat_w = topk_w.reshape(-1)
    order = np.argsort(-flat_w, kind="stable")
    rank = np.zeros(B * K, np.int32)
    counts = np.zeros(E, np.int32)
    for i in order:
        e_ = flat_exp[i]
        rank[i] = counts[e_]
        counts[e_] += 1
    kept = rank < CAP
    kept_w = (topk_w * kept.reshape(B, K)).astype(np.float32)
    kept_w = kept_w / (kept_w.sum(-1, keepdims=True) + np.float32(EPS))
    kept_w_flat = kept_w.reshape(-1)
    slot = np.where(kept, rank, CAP)
    tok_slot = np.zeros((E, CAP + 1), np.int32)
    w_slot = np.zeros((E, CAP + 1), np.float32)
    tok_slot[flat_exp, slot] = flat_tok
    w_slot[flat_exp, slot] = kept_w_flat
    tok_slot = tok_slot[:, :CAP]
    w_slot = w_slot[:, :CAP]
    assigned = np.zeros((B, E), np.float32)
    np.add.at(assigned, (flat_tok, flat_exp), kept.astype(np.float32))
    importance = probs.sum(0)
    load = assigned.sum(0)
    imp_n = importance / (importance.sum() + np.float32(EPS))
    load_n = load / (load.sum() + np.float32(EPS))
    aux = np.float32(E) * (imp_n * load_n).sum()
    return tok_slot, w_slot, np.asarray(aux, dtype=np.float32)


def _combine(dev_out, tok_slot):
    """Scatter-add device outputs [E,CAP,64,C] back to tokens via jax-CPU."""
    import jax
    import jax.numpy as jnp

    if _cpu_device() is None:
        y = np.zeros((B, N * L2, C), np.float32)
        np.add.at(y, tok_slot.reshape(-1), dev_out.reshape(E * CAP, N * L2, C))
        return y.reshape(B, N, L2, C)

    with jax.default_device(_cpu_device()):
        y = (
            jnp.zeros((B, N * L2, C), jnp.float32)
            .at[jnp.asarray(tok_slot.reshape(-1))]
            .add(jnp.asarray(dev_out.reshape(E * CAP, N * L2, C)))
        )
    return np.asarray(y).reshape(B, N, L2, C)


def _shard_inputs(node_embedding, w1, b1, w2, b2, tok_slot, w_slot):
    """Build per-core device input dicts (bf16 matmul operands)."""
    import ml_dtypes

    bf16 = ml_dtypes.bfloat16
    ne = np.ascontiguousarray(node_embedding, dtype=np.float32).reshape(B, N * L2, C)
    w1 = np.asarray(w1, dtype=np.float32)
    b1 = np.asarray(b1, dtype=np.float32)
    w2 = np.asarray(w2, dtype=np.float32)
    b2 = np.asarray(b2, dtype=np.float32)

    in_maps = []
    for c in range(NCORES):
        es = slice(c * EPC, (c + 1) * EPC)
        idx = tok_slot[es].reshape(-1)                       # [EPC*CAP]
        x = ne[idx]                                          # [EPC*CAP, 64, C]
        # [EPC, 128(part), ck, ROWS] with partition p = channel within chunk
        xt = (
            x.reshape(EPC, ROWS, 2, 128)
            .transpose(0, 3, 2, 1)
            .reshape(EPC, 128, 2 * ROWS)
            .astype(bf16)
        )
        # [EPC, 128(part=c within ck), ck*512 + h]
        w1t = (
            w1[es]
            .reshape(EPC, 2, 128, H)
            .transpose(0, 2, 1, 3)
            .reshape(EPC, 128, 2 * H)
            .astype(bf16)
        )
        # [EPC, 128(part=h within hk), hk*256 + c]
        w2t = (
            w2[es]
            .reshape(EPC, 4, 128, C)
            .transpose(0, 2, 1, 3)
            .reshape(EPC, 128, 4 * C)
            .astype(bf16)
        )
        b1t = np.ascontiguousarray(
            b1[es].reshape(EPC, 4, 128).transpose(2, 0, 1).reshape(128, EPC * 4)
        )
        in_maps.append(
            {
                "xt": np.ascontiguousarray(xt),
                "w1t": np.ascontiguousarray(w1t),
                "w2t": np.ascontiguousarray(w2t),
                "b1t": b1t,
            }
        )
    return in_maps


def _build_nc():
    """Per-core Bass program: 16-expert grouped MLP with combine-weight scale."""
    import concourse.bacc as bacc
    import concourse.mybir as mybir
    from concourse.tile import TileContext

    f32 = mybir.dt.float32
    bf16 = mybir.dt.bfloat16

    nc = bacc.Bacc()
    xt_d = nc.dram_tensor("xt", [EPC, 128, 2 * ROWS], bf16, kind="ExternalInput")
    w1_d = nc.dram_tensor("w1t", [EPC, 128, 2 * H], bf16, kind="ExternalInput")
    w2_d = nc.dram_tensor("w2t", [EPC, 128, 4 * C], bf16, kind="ExternalInput")
    b1_d = nc.dram_tensor("b1t", [128, EPC * 4], f32, kind="ExternalInput")
    out_d = nc.dram_tensor("out", [EPC, 128, 2 * ROWS], f32, kind="ExternalOutput")

    with TileContext(nc) as tc:
        with (
            tc.tile_pool(name="xp", bufs=3) as xp,
            tc.tile_pool(name="wp", bufs=3) as wp,
            tc.tile_pool(name="cp", bufs=3) as cp,
            tc.tile_pool(name="hp", bufs=3) as hp,
            tc.tile_pool(name="op", bufs=3) as op_,
            tc.tile_pool(name="pp1", bufs=5, space="PSUM") as pp1,
            tc.tile_pool(name="pp2", bufs=3, space="PSUM") as pp2,
        ):
            b1all = cp.tile([128, EPC * 4], f32, tag="b1", name="b1all", bufs=1)
            nc.sync.dma_start(out=b1all[:], in_=b1_d[:])

            for e in range(EPC):
                xsb = xp.tile([128, 2 * ROWS], bf16, tag="x", name=f"x_{e}")
                w1sb = wp.tile([128, 2 * H], bf16, tag="w1", name=f"w1_{e}")
                w2sb = wp.tile([128, 4 * C], bf16, tag="w2", name=f"w2_{e}")
                if e == 0:
                    hw = ROWS // 2
                    nc.sync.dma_start(out=w1sb[:, :H], in_=w1_d[e][:, :H])
                    nc.sync.dma_start(out=xsb[:, :hw], in_=xt_d[e][:, :hw])
                    nc.sync.dma_start(out=w1sb[:, H:], in_=w1_d[e][:, H:])
                    nc.sync.dma_start(
                        out=xsb[:, ROWS : ROWS + hw], in_=xt_d[e][:, ROWS : ROWS + hw]
                    )
                    nc.sync.dma_start(out=xsb[:, hw:ROWS], in_=xt_d[e][:, hw:ROWS])
                    nc.sync.dma_start(
                        out=xsb[:, ROWS + hw :], in_=xt_d[e][:, ROWS + hw :]
                    )
                    nc.sync.dma_start(out=w2sb[:], in_=w2_d[e])
                else:
                    nc.sync.dma_start(out=w1sb[:], in_=w1_d[e])
                    nc.sync.dma_start(out=xsb[:], in_=xt_d[e])
                    nc.sync.dma_start(out=w2sb[:], in_=w2_d[e])
                osb = op_.tile([128, 2 * ROWS], f32, tag="osb", name=f"osb_{e}")

                for t, (r0, rt) in enumerate(ROW_TILES):
                    ht = hp.tile([128, 4 * rt], bf16, tag="ht", name=f"ht_{e}_{t}")
                    for hk in range(4):
                        ps1 = pp1.tile(
                            [128, rt], f32, tag="ps1", name=f"ps1_{e}_{t}_{hk}"
                        )
                        for ck in range(2):
                            nc.tensor.matmul(
                                ps1[:],
                                lhsT=w1sb[:, ck * H + hk * 128 : ck * H + (hk + 1) * 128],
                                rhs=xsb[:, ck * ROWS + r0 : ck * ROWS + r0 + rt],
                                start=(ck == 0),
                                stop=(ck == 1),
                            )
                        nc.scalar.activation(
                            out=ht[:, hk * rt : (hk + 1) * rt],
                            in_=ps1[:],
                            func=mybir.ActivationFunctionType.Silu,
                            bias=b1all[:, e * 4 + hk : e * 4 + hk + 1],
                        )
                    # gemm2 transposed: out^T[c, rows] = w2^T @ h^T, bias+scale on host
                    for cc in range(2):
                        ps2 = pp2.tile([128, rt], f32, tag="ps2", name=f"ps2_{e}_{t}_{cc}")
                        for hk in range(4):
                            nc.tensor.matmul(
                                ps2[:],
                                lhsT=w2sb[:, hk * C + cc * 128 : hk * C + (cc + 1) * 128],
                                rhs=ht[:, hk * rt : (hk + 1) * rt],
                                start=(hk == 0),
                                stop=(hk == 3),
                            )
                        nc.vector.tensor_copy(
                            osb[:, cc * ROWS + r0 : cc * ROWS + r0 + rt], ps2[:]
                        )
                        if e == EPC - 1:
                            nc.sync.dma_start(
                                out=out_d[e][:, cc * ROWS + r0 : cc * ROWS + r0 + rt],
                                in_=osb[:, cc * ROWS + r0 : cc * ROWS + r0 + rt],
                            )
                if e < EPC - 1:
                    nc.sync.dma_start(out=out_d[e][:, :ROWS], in_=osb[:, :ROWS])
                    nc.sync.dma_start(out=out_d[e][:, ROWS:], in_=osb[:, ROWS:])

    nc.finalize()
    return nc


def _get_nc():
    if "nc" not in _NC_CACHE:
        _NC_CACHE["nc"] = _build_nc()
    return _NC_CACHE["nc"]


def _run_device(in_maps, trace=False, tmpdir=None):
    from concourse.bass_utils import run_bass_kernel_spmd

    nc = _get_nc()
    try:
        res = run_bass_kernel_spmd(
            nc, in_maps, list(range(NCORES)), trace=trace, tmpdir=tmpdir
        )
    except (ImportError, ModuleNotFoundError):
        # BASS_TRACE set in the environment but the axon NTFF hook module is
        # absent -- rerun with tracing hard-disabled.
        os.environ["BASS_NEVER_TRACE"] = "1"
        res = run_bass_kernel_spmd(
            nc, in_maps, list(range(NCORES)), trace=False, tmpdir=tmpdir
        )
    return res


def kernel(node_embedding, router_fea, router_w, w1, b1, w2, b2):
    tok_slot, w_slot, aux = _routing(router_fea, router_w)
    in_maps = _shard_inputs(node_embedding, w1, b1, w2, b2, tok_slot, w_slot)
    res = _run_device(in_maps)
    b2f = np.asarray(b2, dtype=np.float32)
    wfull = np.repeat(w_slot, N * L2, axis=1)  # [E, ROWS]
    parts = []
    for c in range(NCORES):
        o = (
            res.results[c]["out"]
            .reshape(EPC, 128, 2, ROWS)
            .transpose(0, 3, 2, 1)
            .reshape(EPC, ROWS, C)
        )  # [EPC, ROWS, C] raw h@w2 (no bias/scale)
        es = slice(c * EPC, (c + 1) * EPC)
        o = (o + b2f[es][:, None, :]) * wfull[es][:, :, None]
        parts.append(o.reshape(EPC, CAP, N * L2, C))
    dev_out = np.concatenate(parts, axis=0)
    y = _combine(dev_out, tok_slot)
    return y, aux
